# revision 20
# baseline (speedup 1.0000x reference)
"""AttnDecoderRNN step kernel for 8 Trainium2 NeuronCores.

Sharding: vocab dim of out_W sharded across cores (each core computes 6250
logit columns); GRU gate dim + attention feature dim sharded across cores,
reassembled with 2 AllGathers (h0, h1) and 2 AllReduces (scores, concat
pre-activation).

Host prep (numpy): embedding gather of the 64 needed rows, weight
transposes into partition-major packed layouts (so every device DMA is a
few fat contiguous descriptors), bf16 downcast of the streaming-dominant
tensors (out_W, encoder slices).
"""
import os
import sys

sys.path.insert(0, "/opt/trn_rl_repo")

import numpy as np

import concourse.bass as bass
import concourse.mybir as mybir
import concourse.tile as tile
from concourse.tile import add_dep_helper as _adh


def add_dep(from_inst, to_inst, sync=False, reason="order"):
    gi = lambda x: x.ins if hasattr(x, "ins") else x
    _adh(gi(from_inst), gi(to_inst), sync=sync, reason=reason)
from concourse import bacc
from concourse.bass_utils import run_bass_kernel_spmd
from concourse.masks import make_identity

B, H, V, S, L = 64, 1024, 50000, 100, 2
NCORES = 8
HC = H // NCORES        # 128
VC = V // NCORES        # 6250
KT = H // 128           # 8
F32 = mybir.dt.float32
BF16 = mybir.dt.bfloat16
NP_BF16 = np.dtype("bfloat16")

# wide DMA blocks of the out_W shard; each holds 8 k-tiles of [128, WIDE]
WIDE = 2048
WBLOCKS = []
_o = 0
while _o < VC:
    WBLOCKS.append((_o, min(WIDE, VC - _o)))
    _o += WIDE
WT_BUFS = 25

SH = S // 2             # enc1 half (s dim)
JH = HC // 2            # enc2 half (j dim)


def _build():
    nc = bacc.Bacc("TRN2", target_bir_lowering=False, debug=False,
                   num_devices=NCORES)
    f = F32
    inp = {}

    def ein(name, shape, dtype=f):
        inp[name] = nc.dram_tensor(name, shape, dtype, kind="ExternalInput")
        return inp[name]

    # partition-major packed inputs
    ein("xp", [128, KT, B], BF16)
    ein("hp0p", [128, KT, B], BF16); ein("hp1p", [128, KT, B], BF16)
    ein("hp0c", [HC, B]); ein("hp1c", [HC, B])
    ein("wih0", [128, 3, KT, HC], BF16); ein("whh0", [128, 3, KT, HC], BF16)
    ein("wih1", [128, 3, KT, HC], BF16); ein("whh1", [128, 3, KT, HC], BF16)
    ein("b0", [HC, 4]); ein("b1", [HC, 4])
    ein("awcp", [128, KT, HC], BF16)
    ein("enc1", [B, S, HC], BF16)    # enc[:,:,jc].transpose(1,0,2)
    ein("enc2", [B, HC, S], BF16)    # enc[:,:,jc].transpose(1,2,0)
    ein("cwp", [128, 2, KT, HC])     # concat_W.T rows {jc, H+jc} x out-tiles
    ein("cbp", [HC, KT])
    ein("outwt", [H, VC], BF16)

    o_log = nc.dram_tensor("o_logits", [B, VC], f, kind="ExternalOutput")
    o_h0 = nc.dram_tensor("o_h0T", [H, B], f, kind="ExternalOutput")
    o_h1 = nc.dram_tensor("o_h1T", [H, B], f, kind="ExternalOutput")
    o_aw = nc.dram_tensor("o_attnw", [B, S], f, kind="ExternalOutput")

    AX = mybir.AxisListType
    OP = mybir.AluOpType
    ACT = mybir.ActivationFunctionType

    with tile.TileContext(nc) as tc:
        with (
            tc.tile_pool(name="sbp", bufs=1) as sbp,
            tc.tile_pool(name="encp", bufs=2) as encp,
            tc.tile_pool(name="wtp", bufs=WT_BUFS) as wtp,
            tc.tile_pool(name="outp", bufs=2) as outp,
            tc.tile_pool(name="dram", bufs=1, space="DRAM") as dram,
        ):
            # ---------- persistent SBUF loads (sync engine, 1 fat DMA each) ----------
            def pload(name, shape, src=None, dtype=f):
                t = sbp.tile(shape, dtype, name=name + "_sb")
                nc.sync.dma_start(t[:], (src if src is not None
                                         else inp[name].ap()))
                return t

            # GRU weights loaded per gate (r first) so GRU0's first psum
            # group can start as soon as its own 256 KB arrives
            def gload(nm, g):
                t = sbp.tile([128, KT, HC], BF16, name=f"{nm}_{g}_sb")
                nc.sync.dma_start(t[:], inp[nm].ap()[:, g])
                return t

            gw = {nm: [None] * 3 for nm in ("wih0", "whh0", "wih1", "whh1")}
            gw["whh0"][0] = gload("whh0", 0)
            hp0p = pload("hp0p", [128, KT, B], dtype=BF16)
            gw["wih0"][0] = gload("wih0", 0)
            xp = pload("xp", [128, KT, B], dtype=BF16)
            for g in (1, 2):
                gw["whh0"][g] = gload("whh0", g)
                gw["wih0"][g] = gload("wih0", g)
            b0 = pload("b0", [HC, 4])
            hp0c = pload("hp0c", [HC, B])
            for g in range(3):
                gw["whh1"][g] = gload("whh1", g)
            hp1p = pload("hp1p", [128, KT, B], dtype=BF16)
            for g in range(3):
                gw["wih1"][g] = gload("wih1", g)
            hp1c = pload("hp1c", [HC, B])
            b1 = pload("b1", [HC, 4])
            awcp = pload("awcp", [128, KT, HC], dtype=BF16)
            cwp = pload("cwp", [128, 2, KT, HC])
            cbp = pload("cbp", [HC, KT])

            # warm the scalar-engine activation tables off the critical path
            warm = sbp.tile([1, 1], f, name="warm_sb")
            nc.gpsimd.memset(warm[:], 0.0)
            for fn in (ACT.Sigmoid, ACT.Tanh, ACT.Exp, ACT.Identity):
                nc.scalar.activation(warm[:], warm[:], fn)

            ident = sbp.tile([128, 128], f, name="ident_sb")
            make_identity(nc, ident[:])

            # DRAM bounce buffers
            bn0i = dram.tile([HC, B], BF16, name="bn0i")
            bn0o = dram.tile([H, B], BF16, addr_space="Shared", name="bn0o")
            bn1i = dram.tile([HC, B], BF16, name="bn1i")
            bn1o = dram.tile([H, B], BF16, addr_space="Shared", name="bn1o")
            bnsi = dram.tile([128, SH], f, name="bnsi")
            bnso = dram.tile([128, SH], f, addr_space="Shared", name="bnso")
            bnpi = dram.tile([128, KT * B], f, name="bnpi")
            bnpo = dram.tile([128, KT * B], f, addr_space="Shared",
                             name="bnpo")

            rg = [list(range(NCORES))]

            # ---------- out_W stream on gpsimd, throttled ----------
            # Issuing all loads at once floods the (FIFO, priority-less) DMA
            # queues and delays the latency-critical collective bounce
            # writes/reloads by the full backlog. Issue in small batches tied
            # to phase progress instead: a bit up front (overlaps inter-core
            # launch skew), then a batch after each collective trigger.
            wt_tiles = {}
            wt_pending = []
            for wi, (woff, wnb) in enumerate(WBLOCKS):
                for k in range(KT):
                    wt = wtp.tile([128, WIDE], BF16, tag="wt",
                                  name=f"wt_{wi}_{k}")
                    wt_pending.append(
                        (wt, wnb,
                         inp["outwt"].ap()[k * 128:(k + 1) * 128,
                                           woff:woff + wnb]))
                    wt_tiles[(wi, k)] = wt

            def issue_wt(n, anchor=None, eng=None, sync=False):
                for i in range(min(n, len(wt_pending))):
                    wt, wnb, srcap = wt_pending.pop(0)
                    e = eng if eng is not None else nc.sync
                    inst = e.dma_start(wt[:, :wnb], srcap)
                    if anchor is not None:
                        add_dep(inst, anchor, sync=sync,
                                reason="pace out_W stream")

            # ---------- GRU layers ----------
            def gru_layer(lname, x_tiles, hT_tiles, hc_tile, wih, whh, bias,
                          bn_in, bn_out, out_ext, hout_name, after_ag=None):
                with tc.tile_pool(name=f"ps_{lname}", bufs=1,
                                  space="PSUM") as ps:
                    p_r = ps.tile([128, B], f, name=f"{lname}_pr")
                    p_z = ps.tile([128, B], f, name=f"{lname}_pz")
                    p_ni = ps.tile([128, B], f, name=f"{lname}_pni")
                    p_nh = ps.tile([128, B], f, name=f"{lname}_pnh")
                    # h-side (Whh) first: for layer 1 it depends only on
                    # last_hidden, so it overlaps the h0 AllGather wait
                    for g, psum in ((0, p_r), (1, p_z)):
                        for k in range(KT):
                            nc.tensor.matmul(psum[:], whh[g][:, k, :],
                                             hT_tiles[:, k, :],
                                             start=(k == 0), stop=False)
                    for k in range(KT):
                        nc.tensor.matmul(p_nh[:], whh[2][:, k, :],
                                         hT_tiles[:, k, :],
                                         start=(k == 0), stop=(k == KT - 1))
                    for g, psum in ((0, p_r), (1, p_z)):
                        for k in range(KT):
                            nc.tensor.matmul(psum[:], wih[g][:, k, :],
                                             x_tiles[:, k, :],
                                             start=False, stop=(k == KT - 1))
                    for k in range(KT):
                        nc.tensor.matmul(p_ni[:], wih[2][:, k, :],
                                         x_tiles[:, k, :],
                                         start=(k == 0), stop=(k == KT - 1))

                    r = sbp.tile([128, B], f, tag="g_r", name=f"{lname}_r")
                    nc.scalar.activation(r[:], p_r[:], ACT.Sigmoid,
                                         bias=bias[:, 0:1])
                    z = sbp.tile([128, B], f, tag="g_z", name=f"{lname}_z")
                    nc.scalar.activation(z[:], p_z[:], ACT.Sigmoid,
                                         bias=bias[:, 1:2])
                    hn = sbp.tile([128, B], f, tag="g_hn", name=f"{lname}_hn")
                    nc.scalar.activation(hn[:], p_nh[:], ACT.Identity,
                                         bias=bias[:, 3:4])
                    rh = sbp.tile([128, B], f, tag="g_rh", name=f"{lname}_rh")
                    nc.vector.tensor_tensor(rh[:], r[:], hn[:], OP.mult)
                    tn = sbp.tile([128, B], f, tag="g_tn", name=f"{lname}_tn")
                    nc.vector.tensor_tensor(tn[:], p_ni[:], rh[:], OP.add)
                    n = sbp.tile([128, B], f, tag="g_n", name=f"{lname}_n")
                    nc.scalar.activation(n[:], tn[:], ACT.Tanh,
                                         bias=bias[:, 2:3])
                    d = sbp.tile([128, B], f, tag="g_d", name=f"{lname}_d")
                    nc.vector.tensor_tensor(d[:], hc_tile[:], n[:], OP.subtract)
                    zd = sbp.tile([128, B], f, tag="g_zd", name=f"{lname}_zd")
                    nc.vector.tensor_tensor(zd[:], z[:], d[:], OP.mult)
                    hc = sbp.tile([128, B], f, name=f"{lname}_hc")
                    nc.vector.tensor_tensor(hc[:], n[:], zd[:], OP.add)

                hcb = sbp.tile([128, B], BF16, name=f"{lname}_hcb")
                nc.vector.tensor_copy(hcb[:], hc[:])
                bi_inst = nc.sync.dma_start(bn_in[:], hcb[:])
                nc.gpsimd.collective_compute(
                    "AllGather", OP.bypass, replica_groups=rg,
                    ins=[bn_in.opt()], outs=[bn_out.opt()])
                if after_ag is not None:
                    after_ag(bi_inst)
                hT = sbp.tile([128, KT, B], BF16, name=hout_name)
                rl_inst = nc.sync.dma_start(
                    hT[:], bn_out[:].rearrange("(k p) b -> p k b", p=128))
                # f32 upcast of the gathered hidden state, output path only
                hTf = sbp.tile([128, KT, B], f, name=hout_name + "_f")
                nc.vector.tensor_copy(hTf[:], hT[:])
                nc.scalar.dma_start(
                    out_ext.ap().rearrange("(k p) b -> p k b", p=128), hTf[:])
                return hT, hc, bi_inst, rl_inst

            enc_loads = []

            def _after_ag0(bi_inst):
                # enc slices stream during the AG0 window, after the bounce
                for half in range(2):
                    i1 = nc.sync.dma_start(
                        e1d[half * B:(half + 1) * B],
                        inp["enc1"].ap()[:, half * SH:(half + 1) * SH, :])
                    i2 = nc.sync.dma_start(
                        e2d[half * B:(half + 1) * B],
                        inp["enc2"].ap()[:, half * JH:(half + 1) * JH, :])
                    for i in (i1, i2):
                        add_dep(i, bi_inst, reason="enc after bounce write")
                    enc_loads.append((i1, i2))

            e1d = encp.tile([128, SH, HC], BF16, tag="ench", name="e1d")
            e2d = encp.tile([128, JH, S], BF16, tag="ench", name="e2d")
            h0T, _hc0, _, rl0 = gru_layer(
                "g0", xp, hp0p, hp0c, gw["wih0"], gw["whh0"],
                b0, bn0i, bn0o, o_h0, "h0T_sb", after_ag=_after_ag0)
            issue_wt(4, anchor=rl0)
            h1T, hc1, bi1, rl1 = gru_layer(
                "g1", h0T, hp1p, hp1c, gw["wih1"], gw["whh1"],
                b1, bn1i, bn1o, o_h1, "h1T_sb",
                after_ag=lambda bi: issue_wt(4, anchor=bi))
            issue_wt(3, anchor=rl1)

            # ---------- attention ----------
            # (h,b)-split: two 50-s (resp 64-j) halves stacked on the
            # partition axis so the elementwise/reduce work uses all 128
            # DVE lanes instead of 64.
            with tc.tile_pool(name="ps_attn", bufs=1, space="PSUM") as pa, \
                 tc.tile_pool(name="ps_cc", bufs=2, space="PSUM") as pcc_pool:
                # q chunk [B, HC], then duplicated onto both partition halves
                p_q = pa.tile([B, HC], f, name="p_q")
                for k in range(KT):
                    nc.tensor.matmul(p_q[:], h1T[:, k, :], awcp[:, k, :],
                                     start=(k == 0), stop=(k == KT - 1))
                qb2 = sbp.tile([128, HC], BF16, name="qb2_sb")
                nc.vector.tensor_copy(qb2[0:B, :], p_q[:])
                nc.vector.tensor_copy(qb2[B:2 * B, :], p_q[:])

                # scores partial [(h,b), s']: contract j chunk on DVE
                nc.vector.tensor_tensor(
                    e1d[:], e1d[:],
                    qb2[:, None, :].broadcast_to((128, SH, HC)), OP.mult)
                sc2 = sbp.tile([128, SH], f, name="sc2_sb")
                nc.vector.tensor_reduce(sc2[:], e1d[:], axis=AX.X, op=OP.add)

                si_inst = nc.sync.dma_start(bnsi[:], sc2[:])
                nc.gpsimd.collective_compute(
                    "AllReduce", OP.add, replica_groups=rg,
                    ins=[bnsi.opt()], outs=[bnso.opt()])
                issue_wt(4, anchor=si_inst)
                scf = sbp.tile([B, 2, SH], f, name="scf_sb")
                sr_inst = nc.sync.dma_start(
                    scf[:], bnso[:].rearrange("(h b) s -> b h s", h=2))
                issue_wt(3, anchor=sr_inst)
                scff = scf[:].rearrange("b h s -> b (h s)")

                # softmax over S. attn_b's contribution is constant per b,
                # so it cancels in softmax; |scores| is O(1), no max-sub.
                ssum = sbp.tile([B, 1], f, name="ssum_sb")
                ex = sbp.tile([B, S], f, name="ex_sb")
                nc.scalar.activation(ex[:], scff, ACT.Exp,
                                     accum_out=ssum[:])
                rs = sbp.tile([B, 1], f, name="rs_sb")
                nc.vector.reciprocal(rs[:], ssum[:])
                aw = sbp.tile([B, S], f, name="aw_sb")
                nc.any.tensor_scalar_mul(aw[:], ex[:], rs[:])
                nc.scalar.dma_start(o_aw.ap(), aw[:])

                # duplicate aw onto both partition halves (cast to bf16)
                awb2 = sbp.tile([128, S], BF16, name="awb2_sb")
                nc.vector.tensor_copy(awb2[0:B, :], aw[:])
                nc.vector.tensor_copy(awb2[B:2 * B, :], aw[:])

                # context [(h,b), j']: contract s on DVE
                nc.vector.tensor_tensor(
                    e2d[:], e2d[:],
                    awb2[:, None, :].broadcast_to((128, JH, S)), OP.mult)
                ctx2 = sbp.tile([128, JH], f, name="ctx2_sb")
                nc.vector.tensor_reduce(ctx2[:], e2d[:], axis=AX.X, op=OP.add)

                # ctxT[h*64+j', b] = ctx2[h*64+b, j']: one 128-wide transpose
                # then reassemble the two column halves onto partition halves
                p_t2 = pa.tile([B, 128], f, name="p_t2")
                nc.tensor.transpose(p_t2[:], ctx2[:], ident[:])
                ctxT = sbp.tile([HC, B], f, name="ctxT_sb")
                nc.vector.tensor_copy(ctxT[0:B, :], p_t2[:, 0:B])
                nc.vector.tensor_copy(ctxT[B:2 * B, :], p_t2[:, B:2 * B])

                # concat partial pre-activation: this core's 256 contraction
                # dims (its h1 chunk + its ctx chunk) for ALL 1024 out rows
                P_sb = sbp.tile([128, KT, B], f, name="P_sb")
                for m in range(KT):
                    p_c = pcc_pool.tile([128, B], f, tag="pcc",
                                        name=f"pcc_{m}")
                    nc.tensor.matmul(p_c[:], cwp[:, 0, m, :], hc1[:],
                                     start=True, stop=False)
                    nc.tensor.matmul(p_c[:], cwp[:, 1, m, :], ctxT[:],
                                     start=False, stop=True)
                    # copy with concat_b/8 folded in (summed back by the AR)
                    nc.scalar.activation(P_sb[:, m, :], p_c[:], ACT.Identity,
                                         bias=cbp[:, m:m + 1])

                pi_inst = nc.sync.dma_start(
                    bnpi[:].rearrange("p (m b) -> p m b", m=KT), P_sb[:])
                ar2 = nc.gpsimd.collective_compute(
                    "AllReduce", OP.add, replica_groups=rg,
                    ins=[bnpi.opt()], outs=[bnpo.opt()])
                issue_wt(4, anchor=pi_inst)
                praw = sbp.tile([128, KT * B], f, name="praw_sb")
                pr_inst = nc.sync.dma_start(praw[:], bnpo[:])
                issue_wt(len(wt_pending), anchor=pr_inst, eng=nc.gpsimd,
                         sync=True)

                cTw = sbp.tile([128, KT, B], BF16, name="cTw_sb")
                nc.scalar.activation(
                    cTw[:].rearrange("p k b -> p (k b)"), praw[:], ACT.Tanh)



            # ---------- output projection (vocab shard) ----------
            with tc.tile_pool(name="ps_o", bufs=4, space="PSUM") as po:
                for wi, (woff, wnb) in enumerate(WBLOCKS):
                    vo = 0
                    while vo < wnb:
                        nb = min(512, wnb - vo)
                        p_o = po.tile([B, 512], f, tag="p_o",
                                      name=f"po_{woff}_{vo}")
                        for k in range(KT):
                            nc.tensor.matmul(
                                p_o[:, :nb], cTw[:, k, :],
                                wt_tiles[(wi, k)][:, vo:vo + nb],
                                start=(k == 0), stop=(k == KT - 1))
                        osb = outp.tile([B, 512], f, tag="osb",
                                        name=f"osb_{woff}_{vo}")
                        nc.vector.tensor_copy(osb[:, :nb], p_o[:, :nb])
                        nc.sync.dma_start(
                            o_log.ap()[:, woff + vo:woff + vo + nb],
                            osb[:, :nb])
                        vo += nb

    nc.compile()
    return nc


_NC_CACHE = None


def _get_nc():
    global _NC_CACHE
    if _NC_CACHE is None:
        _NC_CACHE = _build()
    return _NC_CACHE


def _pack_pm(a):
    """[1024, X...] -> [128, 8, X...] partition-major contiguous."""
    return np.ascontiguousarray(
        a.reshape(8, 128, *a.shape[1:]).transpose(1, 0, *range(2, a.ndim + 1)))


def _prep_inputs(input_seq, last_hidden, encoder_outputs, emb,
                 Wih0, Whh0, bih0, bhh0, Wih1, Whh1, bih1, bhh1,
                 attn_W, attn_b, concat_W, concat_b, out_W, out_b):
    a = lambda x: np.asarray(x)
    f = lambda x: np.ascontiguousarray(x, dtype=np.float32)
    bf = lambda x: np.ascontiguousarray(np.asarray(x, dtype=np.float32)
                                        .astype(NP_BF16))

    idx = a(input_seq).astype(np.int64)
    x = a(emb)[idx]                        # [B, H]
    xp = _pack_pm(bf(x.T))
    lh = a(last_hidden)
    hp0p, hp1p = _pack_pm(bf(lh[0].T)), _pack_pm(bf(lh[1].T))

    def gru_slices(W):
        WT = a(W).T                        # [H, 3H]
        out = []
        for c in range(NCORES):
            cols = [WT[:, g * H + c * HC:g * H + (c + 1) * HC]
                    for g in range(3)]
            m = bf(np.stack(cols, axis=1))  # [H, 3, HC]
            pm = np.ascontiguousarray(
                m.reshape(8, 128, 3, HC).transpose(1, 2, 0, 3))
            out.append(pm)                  # [128, 3, 8, HC]
        return out

    wih0s, whh0s = gru_slices(Wih0), gru_slices(Whh0)
    wih1s, whh1s = gru_slices(Wih1), gru_slices(Whh1)

    def bias_slices(bih, bhh):
        bih, bhh = a(bih), a(bhh)
        out = []
        for c in range(NCORES):
            sl = slice(c * HC, (c + 1) * HC)
            cols = np.stack([
                bih[0 * H:][sl] + bhh[0 * H:][sl],
                bih[1 * H:][sl] + bhh[1 * H:][sl],
                bih[2 * H:][sl],
                bhh[2 * H:][sl]], axis=1)
            out.append(f(cols))
        return out

    b0s = bias_slices(bih0, bhh0)
    b1s = bias_slices(bih1, bhh1)

    attn_W = a(attn_W)
    cwT = a(concat_W).T                           # [2H, H]
    cbp = f(a(concat_b).reshape(KT, 128).T / NCORES)  # [128, 8]
    out_W = a(out_W)
    out_b = a(out_b)

    in_maps = []
    for c in range(NCORES):
        jc = slice(c * HC, (c + 1) * HC)
        vc = slice(c * VC, (c + 1) * VC)
        encj = a(encoder_outputs)[:, :, jc]
        cwp = np.stack([cwT[c * HC:(c + 1) * HC, :],
                        cwT[H + c * HC:H + (c + 1) * HC, :]], axis=1)
        m = {
            "xp": xp, "hp0p": hp0p, "hp1p": hp1p,
            "hp0c": f(lh[0].T[jc]), "hp1c": f(lh[1].T[jc]),
            "wih0": wih0s[c], "whh0": whh0s[c],
            "wih1": wih1s[c], "whh1": whh1s[c],
            "b0": b0s[c], "b1": b1s[c],
            "awcp": _pack_pm(bf(attn_W[:, jc])),
            "enc1": bf(encj.transpose(1, 0, 2)),
            "enc2": bf(encj.transpose(1, 2, 0)),
            "cwp": f(cwp.reshape(HC, 2, KT, HC)),
            "cbp": cbp,
            "outwt": bf(out_W[vc].T),
        }
        in_maps.append(m)
    return in_maps


LAST_RESULTS = None


def _enable_tracing():
    """Make trace=True work in this container: synthesize the missing
    antenv.axon_hooks module around the libaxon NTFF C API, and stub the
    (egress-blocked) artifact upload."""
    import types

    try:
        from antenv.axon_hooks import get_axon_ntff_profile_hook  # noqa: F401
    except ImportError:
        hook = None
        try:
            from trn_agent_boot.trn_boot import _ntff_profile_via_ctypes
            hook = _ntff_profile_via_ctypes("/opt/axon/libaxon_pjrt.so")
        except Exception:
            pass
        import antenv
        mod = types.ModuleType("antenv.axon_hooks")
        _h = {"hook": hook}
        mod.get_axon_ntff_profile_hook = lambda: _h["hook"]
        mod.set_axon_ntff_profile_hook = lambda h: _h.__setitem__("hook", h)
        sys.modules["antenv.axon_hooks"] = mod
        antenv.axon_hooks = mod

    import concourse.bass_utils as bu
    if not getattr(bu.upload_artifacts, "_stubbed", False):
        def _noop_upload(tmpdir):
            return tmpdir
        _noop_upload._stubbed = True
        bu.upload_artifacts = _noop_upload


def kernel(**inputs):
    global LAST_RESULTS
    nc = _get_nc()
    in_maps = _prep_inputs(**inputs)
    trace = os.environ.get("BASS_KERNEL_TRACE", "0") == "1"
    if trace:
        _enable_tracing()
    res = run_bass_kernel_spmd(nc, in_maps, core_ids=list(range(NCORES)),
                               trace=trace)
    LAST_RESULTS = res
    r = res.results
    output = np.concatenate([r[c]["o_logits"] for c in range(NCORES)], axis=1)
    output += np.asarray(inputs["out_b"], dtype=np.float32)[None, :]
    hidden = np.stack([r[0]["o_h0T"].T, r[0]["o_h1T"].T], axis=0)
    attn_w = r[0]["o_attnw"].reshape(B, 1, S)
    if res.exec_time_ns is not None:
        print(f"HW exec time: {res.exec_time_ns} ns")
    return (output, hidden, attn_w)


# revision 21
# speedup vs baseline: 1.0452x; 1.0452x over previous
"""AttnDecoderRNN step kernel for 8 Trainium2 NeuronCores.

Sharding: vocab dim of out_W sharded across cores (each core computes 6250
logit columns); GRU gate dim + attention feature dim sharded across cores,
reassembled with 2 AllGathers (h0, h1) and 2 AllReduces (scores, concat
pre-activation).

Host prep (numpy): embedding gather of the 64 needed rows, weight
transposes into partition-major packed layouts (so every device DMA is a
few fat contiguous descriptors), bf16 downcast of the streaming-dominant
tensors (out_W, encoder slices).
"""
import os
import sys

sys.path.insert(0, "/opt/trn_rl_repo")

import numpy as np

import concourse.bass as bass
import concourse.mybir as mybir
import concourse.tile as tile
from concourse.tile import add_dep_helper as _adh


def add_dep(from_inst, to_inst, sync=False, reason="order"):
    gi = lambda x: x.ins if hasattr(x, "ins") else x
    _adh(gi(from_inst), gi(to_inst), sync=sync, reason=reason)
from concourse import bacc
from concourse.bass_utils import run_bass_kernel_spmd
from concourse.masks import make_identity

B, H, V, S, L = 64, 1024, 50000, 100, 2
NCORES = 8
HC = H // NCORES        # 128
VC = V // NCORES        # 6250
KT = H // 128           # 8
F32 = mybir.dt.float32
BF16 = mybir.dt.bfloat16
NP_BF16 = np.dtype("bfloat16")

# wide DMA blocks of the out_W shard; each holds 8 k-tiles of [128, WIDE]
WIDE = 2048
WBLOCKS = []
_o = 0
while _o < VC:
    WBLOCKS.append((_o, min(WIDE, VC - _o)))
    _o += WIDE
WT_BUFS = 25

SH = S // 2             # enc1 half (s dim)
JH = HC // 2            # enc2 half (j dim)


def _build():
    nc = bacc.Bacc("TRN2", target_bir_lowering=False, debug=False,
                   num_devices=NCORES)
    f = F32
    inp = {}

    def ein(name, shape, dtype=f):
        inp[name] = nc.dram_tensor(name, shape, dtype, kind="ExternalInput")
        return inp[name]

    # partition-major packed inputs
    ein("xp", [128, KT, B], BF16)
    ein("hp0p", [128, KT, B], BF16); ein("hp1p", [128, KT, B], BF16)
    ein("hp0c", [HC, B]); ein("hp1c", [HC, B])
    ein("wih0", [128, 3, KT, HC], BF16); ein("whh0", [128, 3, KT, HC], BF16)
    ein("wih1", [128, 3, KT, HC], BF16); ein("whh1", [128, 3, KT, HC], BF16)
    ein("b0", [HC, 4]); ein("b1", [HC, 4])
    ein("awcp", [128, KT, HC], BF16)
    ein("enc1", [B, S, HC], BF16)    # enc[:,:,jc].transpose(1,0,2)
    ein("enc2", [B, HC, S], BF16)    # enc[:,:,jc].transpose(1,2,0)
    ein("cwp0", [128, KT, HC])       # concat_W.T rows jc (this core's h1 chunk)
    ein("cw1p", [128, KT, KT, HC], BF16)   # concat_W.T rows H..2H, packed
    ein("cbp", [HC, KT])
    ein("outwt", [H, VC], BF16)

    o_log = nc.dram_tensor("o_logits", [B, VC], f, kind="ExternalOutput")
    o_h0 = nc.dram_tensor("o_h0T", [H, B], f, kind="ExternalOutput")
    o_h1 = nc.dram_tensor("o_h1T", [H, B], f, kind="ExternalOutput")
    o_aw = nc.dram_tensor("o_attnw", [B, S], f, kind="ExternalOutput")

    AX = mybir.AxisListType
    OP = mybir.AluOpType
    ACT = mybir.ActivationFunctionType

    with tile.TileContext(nc) as tc:
        with (
            tc.tile_pool(name="sbp", bufs=1) as sbp,
            tc.tile_pool(name="encp", bufs=2) as encp,
            tc.tile_pool(name="wtp", bufs=WT_BUFS) as wtp,
            tc.tile_pool(name="outp", bufs=2) as outp,
            tc.tile_pool(name="dram", bufs=1, space="DRAM") as dram,
        ):
            # ---------- persistent SBUF loads (sync engine, 1 fat DMA each) ----------
            def pload(name, shape, src=None, dtype=f):
                t = sbp.tile(shape, dtype, name=name + "_sb")
                nc.sync.dma_start(t[:], (src if src is not None
                                         else inp[name].ap()))
                return t

            # GRU weights loaded per gate (r first) so GRU0's first psum
            # group can start as soon as its own 256 KB arrives
            def gload(nm, g):
                t = sbp.tile([128, KT, HC], BF16, name=f"{nm}_{g}_sb")
                nc.sync.dma_start(t[:], inp[nm].ap()[:, g])
                return t

            gw = {nm: [None] * 3 for nm in ("wih0", "whh0", "wih1", "whh1")}
            gw["whh0"][0] = gload("whh0", 0)
            hp0p = pload("hp0p", [128, KT, B], dtype=BF16)
            gw["wih0"][0] = gload("wih0", 0)
            xp = pload("xp", [128, KT, B], dtype=BF16)
            for g in (1, 2):
                gw["whh0"][g] = gload("whh0", g)
                gw["wih0"][g] = gload("wih0", g)
            b0 = pload("b0", [HC, 4])
            hp0c = pload("hp0c", [HC, B])
            for g in range(3):
                gw["whh1"][g] = gload("whh1", g)
            hp1p = pload("hp1p", [128, KT, B], dtype=BF16)
            for g in range(3):
                gw["wih1"][g] = gload("wih1", g)
            hp1c = pload("hp1c", [HC, B])
            b1 = pload("b1", [HC, 4])
            awcp = pload("awcp", [128, KT, HC], dtype=BF16)
            cwp0 = pload("cwp0", [128, KT, HC])
            cw1p = pload("cw1p", [128, KT, KT, HC], dtype=BF16)
            cbp = pload("cbp", [HC, KT])

            # warm the scalar-engine activation tables off the critical path
            warm = sbp.tile([1, 1], f, name="warm_sb")
            nc.gpsimd.memset(warm[:], 0.0)
            for fn in (ACT.Sigmoid, ACT.Tanh, ACT.Exp, ACT.Identity):
                nc.scalar.activation(warm[:], warm[:], fn)

            ident = sbp.tile([128, 128], f, name="ident_sb")
            make_identity(nc, ident[:])

            # DRAM bounce buffers
            bn0i = dram.tile([HC, B], BF16, name="bn0i")
            bn0o = dram.tile([H, B], BF16, addr_space="Shared", name="bn0o")
            bn1i = dram.tile([HC, B], BF16, name="bn1i")
            bn1o = dram.tile([H, B], BF16, addr_space="Shared", name="bn1o")
            bnsi = dram.tile([128, SH], f, name="bnsi")
            bnso = dram.tile([128, SH], f, addr_space="Shared", name="bnso")
            bnpi = dram.tile([128, KT * B], f, name="bnpi")
            bnpo = dram.tile([128, KT * B], f, addr_space="Shared",
                             name="bnpo")
            bnci = dram.tile([HC, B], BF16, name="bnci")
            bnco = dram.tile([H, B], BF16, addr_space="Shared", name="bnco")

            rg = [list(range(NCORES))]

            # ---------- out_W stream on gpsimd, throttled ----------
            # Issuing all loads at once floods the (FIFO, priority-less) DMA
            # queues and delays the latency-critical collective bounce
            # writes/reloads by the full backlog. Issue in small batches tied
            # to phase progress instead: a bit up front (overlaps inter-core
            # launch skew), then a batch after each collective trigger.
            wt_tiles = {}
            wt_pending = []
            for wi, (woff, wnb) in enumerate(WBLOCKS):
                for k in range(KT):
                    wt = wtp.tile([128, WIDE], BF16, tag="wt",
                                  name=f"wt_{wi}_{k}")
                    wt_pending.append(
                        (wt, wnb,
                         inp["outwt"].ap()[k * 128:(k + 1) * 128,
                                           woff:woff + wnb]))
                    wt_tiles[(wi, k)] = wt

            def issue_wt(n, anchor=None, eng=None, sync=False):
                for i in range(min(n, len(wt_pending))):
                    wt, wnb, srcap = wt_pending.pop(0)
                    e = eng if eng is not None else nc.sync
                    inst = e.dma_start(wt[:, :wnb], srcap)
                    if anchor is not None:
                        add_dep(inst, anchor, sync=sync,
                                reason="pace out_W stream")

            # ---------- GRU layers ----------
            def gru_layer(lname, x_tiles, hT_tiles, hc_tile, wih, whh, bias,
                          bn_in, bn_out, out_ext, hout_name, after_ag=None):
                with tc.tile_pool(name=f"ps_{lname}", bufs=1,
                                  space="PSUM") as ps:
                    p_r = ps.tile([128, B], f, name=f"{lname}_pr")
                    p_z = ps.tile([128, B], f, name=f"{lname}_pz")
                    p_ni = ps.tile([128, B], f, name=f"{lname}_pni")
                    p_nh = ps.tile([128, B], f, name=f"{lname}_pnh")
                    # h-side (Whh) first: for layer 1 it depends only on
                    # last_hidden, so it overlaps the h0 AllGather wait
                    for g, psum in ((0, p_r), (1, p_z)):
                        for k in range(KT):
                            nc.tensor.matmul(psum[:], whh[g][:, k, :],
                                             hT_tiles[:, k, :],
                                             start=(k == 0), stop=False)
                    for k in range(KT):
                        nc.tensor.matmul(p_nh[:], whh[2][:, k, :],
                                         hT_tiles[:, k, :],
                                         start=(k == 0), stop=(k == KT - 1))
                    for g, psum in ((0, p_r), (1, p_z)):
                        for k in range(KT):
                            nc.tensor.matmul(psum[:], wih[g][:, k, :],
                                             x_tiles[:, k, :],
                                             start=False, stop=(k == KT - 1))
                    for k in range(KT):
                        nc.tensor.matmul(p_ni[:], wih[2][:, k, :],
                                         x_tiles[:, k, :],
                                         start=(k == 0), stop=(k == KT - 1))

                    r = sbp.tile([128, B], f, tag="g_r", name=f"{lname}_r")
                    nc.scalar.activation(r[:], p_r[:], ACT.Sigmoid,
                                         bias=bias[:, 0:1])
                    z = sbp.tile([128, B], f, tag="g_z", name=f"{lname}_z")
                    nc.scalar.activation(z[:], p_z[:], ACT.Sigmoid,
                                         bias=bias[:, 1:2])
                    hn = sbp.tile([128, B], f, tag="g_hn", name=f"{lname}_hn")
                    nc.scalar.activation(hn[:], p_nh[:], ACT.Identity,
                                         bias=bias[:, 3:4])
                    rh = sbp.tile([128, B], f, tag="g_rh", name=f"{lname}_rh")
                    nc.vector.tensor_tensor(rh[:], r[:], hn[:], OP.mult)
                    tn = sbp.tile([128, B], f, tag="g_tn", name=f"{lname}_tn")
                    nc.vector.tensor_tensor(tn[:], p_ni[:], rh[:], OP.add)
                    n = sbp.tile([128, B], f, tag="g_n", name=f"{lname}_n")
                    nc.scalar.activation(n[:], tn[:], ACT.Tanh,
                                         bias=bias[:, 2:3])
                    d = sbp.tile([128, B], f, tag="g_d", name=f"{lname}_d")
                    nc.vector.tensor_tensor(d[:], hc_tile[:], n[:], OP.subtract)
                    zd = sbp.tile([128, B], f, tag="g_zd", name=f"{lname}_zd")
                    nc.vector.tensor_tensor(zd[:], z[:], d[:], OP.mult)
                    hc = sbp.tile([128, B], f, name=f"{lname}_hc")
                    nc.vector.tensor_tensor(hc[:], n[:], zd[:], OP.add)

                hcb = sbp.tile([128, B], BF16, name=f"{lname}_hcb")
                nc.vector.tensor_copy(hcb[:], hc[:])
                bi_inst = nc.sync.dma_start(bn_in[:], hcb[:])
                nc.gpsimd.collective_compute(
                    "AllGather", OP.bypass, replica_groups=rg,
                    ins=[bn_in.opt()], outs=[bn_out.opt()])
                if after_ag is not None:
                    after_ag(bi_inst)
                hT = sbp.tile([128, KT, B], BF16, name=hout_name)
                rl_inst = nc.sync.dma_start(
                    hT[:], bn_out[:].rearrange("(k p) b -> p k b", p=128))
                # f32 upcast of the gathered hidden state, output path only
                hTf = sbp.tile([128, KT, B], f, name=hout_name + "_f")
                nc.vector.tensor_copy(hTf[:], hT[:])
                nc.scalar.dma_start(
                    out_ext.ap().rearrange("(k p) b -> p k b", p=128), hTf[:])
                return hT, hc, bi_inst, rl_inst

            enc_loads = []

            def _after_ag0(bi_inst):
                # enc slices stream during the AG0 window, after the bounce
                for half in range(2):
                    i1 = nc.sync.dma_start(
                        e1d[half * B:(half + 1) * B],
                        inp["enc1"].ap()[:, half * SH:(half + 1) * SH, :])
                    i2 = nc.sync.dma_start(
                        e2d[half * B:(half + 1) * B],
                        inp["enc2"].ap()[:, half * JH:(half + 1) * JH, :])
                    for i in (i1, i2):
                        add_dep(i, bi_inst, reason="enc after bounce write")
                    enc_loads.append((i1, i2))

            e1d = encp.tile([128, SH, HC], BF16, tag="ench", name="e1d")
            e2d = encp.tile([128, JH, S], BF16, tag="ench", name="e2d")
            h0T, _hc0, _, rl0 = gru_layer(
                "g0", xp, hp0p, hp0c, gw["wih0"], gw["whh0"],
                b0, bn0i, bn0o, o_h0, "h0T_sb", after_ag=_after_ag0)
            issue_wt(4, anchor=rl0)
            h1T, hc1, bi1, rl1 = gru_layer(
                "g1", h0T, hp1p, hp1c, gw["wih1"], gw["whh1"],
                b1, bn1i, bn1o, o_h1, "h1T_sb",
                after_ag=lambda bi: issue_wt(4, anchor=bi))
            issue_wt(3, anchor=rl1)

            # ---------- attention + concat ----------
            with tc.tile_pool(name="ps_attn", bufs=1, space="PSUM") as pa, \
                 tc.tile_pool(name="ps_cc", bufs=2, space="PSUM") as pcc_pool:
                # concat h1-half partials: ready right after GRU1, AllReduce
                # overlaps the whole scores/softmax/context phase
                PA_sb = sbp.tile([128, KT, B], f, name="PA_sb")
                for m in range(KT):
                    p_c = pcc_pool.tile([128, B], f, tag="pcc",
                                        name=f"pcca_{m}")
                    nc.tensor.matmul(p_c[:], cwp0[:, m, :], hc1[:],
                                     start=True, stop=True)
                    # fold concat_b/8 in here (summed back by the AR)
                    nc.scalar.activation(PA_sb[:, m, :], p_c[:], ACT.Identity,
                                         bias=cbp[:, m:m + 1])
                pi_inst = nc.sync.dma_start(
                    bnpi[:].rearrange("p (m b) -> p m b", m=KT), PA_sb[:])
                nc.gpsimd.collective_compute(
                    "AllReduce", OP.add, replica_groups=rg,
                    ins=[bnpi.opt()], outs=[bnpo.opt()])
                praw = sbp.tile([128, KT, B], f, name="praw_sb")
                pr_inst = nc.sync.dma_start(
                    praw[:], bnpo[:].rearrange("p (m b) -> p m b", m=KT))
                issue_wt(4, anchor=pi_inst)

                # q chunk [B, HC], then duplicated onto both partition halves
                p_q = pa.tile([B, HC], f, name="p_q")
                for k in range(KT):
                    nc.tensor.matmul(p_q[:], h1T[:, k, :], awcp[:, k, :],
                                     start=(k == 0), stop=(k == KT - 1))
                qb2 = sbp.tile([128, HC], BF16, name="qb2_sb")
                nc.vector.tensor_copy(qb2[0:B, :], p_q[:])
                nc.vector.tensor_copy(qb2[B:2 * B, :], p_q[:])

                # scores partial [(h,b), s']: contract j chunk on DVE
                nc.vector.tensor_tensor(
                    e1d[:], e1d[:],
                    qb2[:, None, :].broadcast_to((128, SH, HC)), OP.mult)
                sc2 = sbp.tile([128, SH], f, name="sc2_sb")
                nc.vector.tensor_reduce(sc2[:], e1d[:], axis=AX.X, op=OP.add)

                si_inst = nc.sync.dma_start(bnsi[:], sc2[:])
                nc.gpsimd.collective_compute(
                    "AllReduce", OP.add, replica_groups=rg,
                    ins=[bnsi.opt()], outs=[bnso.opt()])
                issue_wt(4, anchor=si_inst)
                scf = sbp.tile([B, 2, SH], f, name="scf_sb")
                sr_inst = nc.sync.dma_start(
                    scf[:], bnso[:].rearrange("(h b) s -> b h s", h=2))
                issue_wt(3, anchor=sr_inst)
                scff = scf[:].rearrange("b h s -> b (h s)")

                # softmax over S. attn_b's contribution is constant per b,
                # so it cancels in softmax; |scores| is O(1), no max-sub.
                ssum = sbp.tile([B, 1], f, name="ssum_sb")
                ex = sbp.tile([B, S], f, name="ex_sb")
                nc.scalar.activation(ex[:], scff, ACT.Exp,
                                     accum_out=ssum[:])
                rs = sbp.tile([B, 1], f, name="rs_sb")
                nc.vector.reciprocal(rs[:], ssum[:])
                aw = sbp.tile([B, S], f, name="aw_sb")
                nc.any.tensor_scalar_mul(aw[:], ex[:], rs[:])
                nc.scalar.dma_start(o_aw.ap(), aw[:])

                # duplicate aw onto both partition halves (cast to bf16)
                awb2 = sbp.tile([128, S], BF16, name="awb2_sb")
                nc.vector.tensor_copy(awb2[0:B, :], aw[:])
                nc.vector.tensor_copy(awb2[B:2 * B, :], aw[:])

                # context [(h,b), j']: contract s on DVE
                nc.vector.tensor_tensor(
                    e2d[:], e2d[:],
                    awb2[:, None, :].broadcast_to((128, JH, S)), OP.mult)
                ctx2 = sbp.tile([128, JH], f, name="ctx2_sb")
                nc.vector.tensor_reduce(ctx2[:], e2d[:], axis=AX.X, op=OP.add)

                # ctxT[h*64+j', b] = ctx2[h*64+b, j']: one 128-wide transpose
                # then reassemble the two column halves onto partition halves
                p_t2 = pa.tile([B, 128], f, name="p_t2")
                nc.tensor.transpose(p_t2[:], ctx2[:], ident[:])
                ctxTb = sbp.tile([HC, B], BF16, name="ctxTb_sb")
                nc.vector.tensor_copy(ctxTb[0:B, :], p_t2[:, 0:B])
                nc.vector.tensor_copy(ctxTb[B:2 * B, :], p_t2[:, B:2 * B])

                # AllGather ctx chunks (bf16), then every core computes the
                # ctx half of the concat contraction locally
                ci_inst = nc.sync.dma_start(bnci[:], ctxTb[:])
                agc = nc.gpsimd.collective_compute(
                    "AllGather", OP.bypass, replica_groups=rg,
                    ins=[bnci.opt()], outs=[bnco.opt()])
                cxT = sbp.tile([128, KT, B], BF16, name="cxT_sb")
                cx_inst = nc.sync.dma_start(
                    cxT[:], bnco[:].rearrange("(k p) b -> p k b", p=128))
                issue_wt(4, anchor=ci_inst)
                issue_wt(len(wt_pending), anchor=agc, eng=nc.gpsimd,
                         sync=True)

                cTw = sbp.tile([128, KT, B], BF16, name="cTw_sb")
                for m in range(KT):
                    p_c2 = pcc_pool.tile([128, B], f, tag="pcc",
                                         name=f"pccb_{m}")
                    for k in range(KT):
                        nc.tensor.matmul(p_c2[:], cw1p[:, k, m, :],
                                         cxT[:, k, :],
                                         start=(k == 0), stop=(k == KT - 1))
                    t_m = sbp.tile([128, B], f, tag="t_m", name=f"tm_{m}")
                    nc.vector.tensor_tensor(t_m[:], p_c2[:], praw[:, m, :],
                                            OP.add)
                    nc.scalar.activation(cTw[:, m, :], t_m[:], ACT.Tanh)

            # ---------- output projection (vocab shard) ----------
            with tc.tile_pool(name="ps_o", bufs=4, space="PSUM") as po:
                for wi, (woff, wnb) in enumerate(WBLOCKS):
                    vo = 0
                    while vo < wnb:
                        nb = min(512, wnb - vo)
                        p_o = po.tile([B, 512], f, tag="p_o",
                                      name=f"po_{woff}_{vo}")
                        for k in range(KT):
                            nc.tensor.matmul(
                                p_o[:, :nb], cTw[:, k, :],
                                wt_tiles[(wi, k)][:, vo:vo + nb],
                                start=(k == 0), stop=(k == KT - 1))
                        osb = outp.tile([B, 512], f, tag="osb",
                                        name=f"osb_{woff}_{vo}")
                        nc.vector.tensor_copy(osb[:, :nb], p_o[:, :nb])
                        nc.sync.dma_start(
                            o_log.ap()[:, woff + vo:woff + vo + nb],
                            osb[:, :nb])
                        vo += nb

    nc.compile()
    return nc


_NC_CACHE = None


def _get_nc():
    global _NC_CACHE
    if _NC_CACHE is None:
        _NC_CACHE = _build()
    return _NC_CACHE


def _pack_pm(a):
    """[1024, X...] -> [128, 8, X...] partition-major contiguous."""
    return np.ascontiguousarray(
        a.reshape(8, 128, *a.shape[1:]).transpose(1, 0, *range(2, a.ndim + 1)))


def _prep_inputs(input_seq, last_hidden, encoder_outputs, emb,
                 Wih0, Whh0, bih0, bhh0, Wih1, Whh1, bih1, bhh1,
                 attn_W, attn_b, concat_W, concat_b, out_W, out_b):
    a = lambda x: np.asarray(x)
    f = lambda x: np.ascontiguousarray(x, dtype=np.float32)
    bf = lambda x: np.ascontiguousarray(np.asarray(x, dtype=np.float32)
                                        .astype(NP_BF16))

    idx = a(input_seq).astype(np.int64)
    x = a(emb)[idx]                        # [B, H]
    xp = _pack_pm(bf(x.T))
    lh = a(last_hidden)
    hp0p, hp1p = _pack_pm(bf(lh[0].T)), _pack_pm(bf(lh[1].T))

    def gru_slices(W):
        WT = a(W).T                        # [H, 3H]
        out = []
        for c in range(NCORES):
            cols = [WT[:, g * H + c * HC:g * H + (c + 1) * HC]
                    for g in range(3)]
            m = bf(np.stack(cols, axis=1))  # [H, 3, HC]
            pm = np.ascontiguousarray(
                m.reshape(8, 128, 3, HC).transpose(1, 2, 0, 3))
            out.append(pm)                  # [128, 3, 8, HC]
        return out

    wih0s, whh0s = gru_slices(Wih0), gru_slices(Whh0)
    wih1s, whh1s = gru_slices(Wih1), gru_slices(Whh1)

    def bias_slices(bih, bhh):
        bih, bhh = a(bih), a(bhh)
        out = []
        for c in range(NCORES):
            sl = slice(c * HC, (c + 1) * HC)
            cols = np.stack([
                bih[0 * H:][sl] + bhh[0 * H:][sl],
                bih[1 * H:][sl] + bhh[1 * H:][sl],
                bih[2 * H:][sl],
                bhh[2 * H:][sl]], axis=1)
            out.append(f(cols))
        return out

    b0s = bias_slices(bih0, bhh0)
    b1s = bias_slices(bih1, bhh1)

    attn_W = a(attn_W)
    cwT = a(concat_W).T                           # [2H, H]
    cw1p_full = np.ascontiguousarray(
        bf(cwT[H:2 * H, :]).reshape(KT, 128, KT, HC)
        .transpose(1, 0, 2, 3))                   # [128, kc, m, HC] bf16
    cbp = f(a(concat_b).reshape(KT, 128).T / NCORES)  # [128, 8]
    out_W = a(out_W)
    out_b = a(out_b)

    in_maps = []
    for c in range(NCORES):
        jc = slice(c * HC, (c + 1) * HC)
        vc = slice(c * VC, (c + 1) * VC)
        encj = a(encoder_outputs)[:, :, jc]
        m = {
            "xp": xp, "hp0p": hp0p, "hp1p": hp1p,
            "hp0c": f(lh[0].T[jc]), "hp1c": f(lh[1].T[jc]),
            "wih0": wih0s[c], "whh0": whh0s[c],
            "wih1": wih1s[c], "whh1": whh1s[c],
            "b0": b0s[c], "b1": b1s[c],
            "awcp": _pack_pm(bf(attn_W[:, jc])),
            "enc1": bf(encj.transpose(1, 0, 2)),
            "enc2": bf(encj.transpose(1, 2, 0)),
            "cwp0": f(cwT[c * HC:(c + 1) * HC, :].reshape(HC, KT, HC)),
            "cw1p": cw1p_full,
            "cbp": cbp,
            "outwt": bf(out_W[vc].T),
        }
        in_maps.append(m)
    return in_maps


LAST_RESULTS = None


def _enable_tracing():
    """Make trace=True work in this container: synthesize the missing
    antenv.axon_hooks module around the libaxon NTFF C API, and stub the
    (egress-blocked) artifact upload."""
    import types

    try:
        from antenv.axon_hooks import get_axon_ntff_profile_hook  # noqa: F401
    except ImportError:
        hook = None
        try:
            from trn_agent_boot.trn_boot import _ntff_profile_via_ctypes
            hook = _ntff_profile_via_ctypes("/opt/axon/libaxon_pjrt.so")
        except Exception:
            pass
        import antenv
        mod = types.ModuleType("antenv.axon_hooks")
        _h = {"hook": hook}
        mod.get_axon_ntff_profile_hook = lambda: _h["hook"]
        mod.set_axon_ntff_profile_hook = lambda h: _h.__setitem__("hook", h)
        sys.modules["antenv.axon_hooks"] = mod
        antenv.axon_hooks = mod

    import concourse.bass_utils as bu
    if not getattr(bu.upload_artifacts, "_stubbed", False):
        def _noop_upload(tmpdir):
            return tmpdir
        _noop_upload._stubbed = True
        bu.upload_artifacts = _noop_upload


def kernel(**inputs):
    global LAST_RESULTS
    nc = _get_nc()
    in_maps = _prep_inputs(**inputs)
    trace = os.environ.get("BASS_KERNEL_TRACE", "0") == "1"
    if trace:
        _enable_tracing()
    res = run_bass_kernel_spmd(nc, in_maps, core_ids=list(range(NCORES)),
                               trace=trace)
    LAST_RESULTS = res
    r = res.results
    output = np.concatenate([r[c]["o_logits"] for c in range(NCORES)], axis=1)
    output += np.asarray(inputs["out_b"], dtype=np.float32)[None, :]
    hidden = np.stack([r[0]["o_h0T"].T, r[0]["o_h1T"].T], axis=0)
    attn_w = r[0]["o_attnw"].reshape(B, 1, S)
    if res.exec_time_ns is not None:
        print(f"HW exec time: {res.exec_time_ns} ns")
    return (output, hidden, attn_w)
